# revision 54
# baseline (speedup 1.0000x reference)
"""DiceLoss Trainium2 Bass kernel, v6 (multi-engine softmax split).

Problem: logits [8, 11, 512, 512] f32, targets [8, 512, 512] int.
  probs = softmax(logits, axis=1); I[c] = sum probs[c]*(t==c);
  Card[c] = sum probs[c] + count(t==c); loss = 1 - mean((2I+1)/(Card+1)).

Sharding: data-parallel over batch; core b handles batch element b
(262144 pixels laid out [128 partitions, 2048 free], processed in 8
chunks of 256 pixel-columns).

Per-pixel work is split across all four compute engines:
  e_c = exp(x_c):
    - 7 blocks (c0..c5, c10): exact exp on ACT (fp8 input)
    - 5 blocks (xt, c7..c9, c6): DVE "bit-hack" exp at 4x DVE speed:
      i16 = rne(x*128*log2e + B1) from bf16 input; bitcast as bf16 is
      (1+f)*2^k ~= 2^z (piecewise-linear 2^frac, mean-centered by B1;
      end-to-end error validated ~1e-4 in numsim.py)
  S = sum_c e_c: PE identity-matmul accumulation into PSUM (11 matmuls
    per chunk, hack blocks first since they land earlier; PE is warmed
    up with dummy matmuls so its pstate ramp completes before S(0))
  r = 1/S: DVE reciprocal
  y_c = e_c * r:
    - c0,c1,c2 on Pool (tensor_tensor, stride-0 broadcast r)
    - c3,c4,c5,c10,xt on DVE (one fused tensor_tensor)
    - c7,c8,c9: y never materialized in mid chunks; SP[c] comes from
      the PE "trace trick": M_c += E_slice^T R_slice accumulated over
      128-col slices; diag(M_c) holds per-column sums of e_c*r, host
      sums the diagonal. The last `ntail` chunks materialize y on DVE
      and use column-sum matmuls instead, so the M banks close early
      and their PSUM copies/DMA overlap the drain; tail-unit y splits
      DVE (9 blocks) / Pool (2 blocks) so the drain runs in parallel.
  SP[c] (c0..c5, c10): PE column-sum matmuls (y-slice stationary, ones
    moving, [128,1] psum out: cost ~ 1 moving row, nearly free)
  SP[c6]: residual (sum_c y_c = 1 per pixel => Npix - sum others)
  u = y[xt] (target-class prob of the host-gathered x_t block): DMA'd
    out in chunk pairs; host computes I[c] = bincount(t, u) and
    CNT[c] = bincount(t) (indexing only) plus the final dice ratio.

Pipelining: per chunk, front = DMA/exp/hack/S, back_dve = recip/y/u-out
(one chunk behind), back_pe = colsum/trace matmuls (two behind) - the
split keeps each engine's in-order queue from stalling the next front.
Input rides one bf16-typed blob ([x8 section | x16 section], fp8 part
DMA'd first per chunk); the M matrices ship early, SP columns at the
end. NOTE: reading a PSUM bank shortly after its accumulation group
stops is racy on this stack (the early-spc variant flaked) - keep
PSUM reads well behind their producing matmuls.
TimelineSim: 26320 ns (baseline v5: 35013 ns); HW rel err 4.3e-5,
bit-stable across processes.
"""

import numpy as np
import ml_dtypes

import concourse.bass as bass
import concourse.tile as tile
from concourse import mybir
from concourse.bass_utils import run_bass_kernel_spmd

B, C, H, W = 8, 11, 512, 512
NP_, NF = 128, 2048             # partition x free pixels per core
SMOOTH = 1.0

FP32 = mybir.dt.float32
BF16 = mybir.dt.bfloat16
FP8 = mybir.dt.float8e4
I16 = mybir.dt.int16
AF = mybir.ActivationFunctionType
ALU = mybir.AluOpType

LOG2E = float(np.log2(np.e))
A1 = 128.0 * LOG2E              # z-scale: i16 units per unit x
MEAN_EXCESS = 0.057304959111036226  # E[log2(1+f)-f], f~U[0,1)
B1 = 16256.0 - 128.0 * MEAN_EXCESS  # centers the 2^frac PWL error

# --- e-tile position map --------------------------------------------
# P0..P2:  c0,c1,c2  e=ACT   y=Pool
# P3..P5:  c3,c4,c5  e=ACT   y=DVE
# P6:      c10       e=ACT   y=DVE
# P7:      xt        e=hack  y=DVE -> u out
# P8..P10: c7,c8,c9  e=hack  SP=PE trace (y on DVE for last chunk)
# P11:     c6        e=hack  residual (no y)
NBLK = 12
N_X8 = 7                        # fp8 blocks: [c0..c5, c10] -> P0..P6
N_X16 = 5                       # bf16 blocks: [xt,c7,c8,c9,c6] -> P7..P11
ACT_SPAN = (0, 7)
H16_SPAN = (7, 12)
POOL_Y = (0, 3)
DVE_Y = (3, 8)                  # includes P6 (c10): its y is computed
                                # but unused in mid units (SP via trace)
N_SPC = 7                       # colsum classes P0..P6 (c10's y exists
                                # anyway inside the fused DVE span)
TRACE_P = [8, 9, 10]            # traced positions (c7, c8, c9)
S_SKIP = 7                      # xt not part of S
P_CLASS = [0, 1, 2, 3, 4, 5, 10, -1, 7, 8, 9, 6]  # class of each position
BPP = N_X8 + 2 * N_X16          # input bytes per pixel-column (17)

CFG = dict(
    ws=[256] * 8,               # front (DMA/exp/hack/S) chunk widths
    units=[1] * 8,              # back-stage units, in chunks
    warmup=46,                  # PE warmup matmuls (128 rows each)
    ntail=2,                    # trailing units using colsums (no trace)
    u_group=2,                  # chunks per u-out DMA
)


def build_nc(**over):
    cfg = dict(CFG)
    cfg.update(over)
    ws = cfg["ws"]
    nh = len(ws)
    assert sum(ws) == NF and all(w % 128 == 0 and w <= 512 for w in ws)
    units = cfg["units"]        # chunks per back-unit
    nu = len(units)
    assert sum(units) == nh
    ub = [sum(units[:u]) for u in range(nu + 1)]   # unit chunk bounds
    unit_of = [u for u in range(nu) for _ in range(units[u])]
    ntail = cfg["ntail"]        # units whose trace classes use colsums
    n_spcols = N_SPC * nu + len(TRACE_P) * ntail

    nc = bass.Bass(trn_type="TRN2")

    # input blob (bf16-typed bytes): [whole x8 section | x16 section],
    # each chunk-major, so adjacent chunks merge into one DMA per pair
    xall_d = nc.declare_dram_parameter("xall", [NP_ * BPP * NF // 2], BF16,
                                       isOutput=False)
    u_d = nc.declare_dram_parameter("u_out", [NP_, NF], BF16, isOutput=True)
    sm_d = nc.declare_dram_parameter(
        "sm_out", [NP_, n_spcols + len(TRACE_P) * NP_], BF16, isOutput=True)

    # single const blob: [ident | ones] -> one DMA
    const_np = np.concatenate(
        [np.eye(NP_, dtype=np.float32), np.ones((NP_, 1), np.float32)],
        axis=1).astype(ml_dtypes.bfloat16)
    const_dram = nc.inline_tensor(const_np, name="constb")

    offs = [sum(ws[:h]) for h in range(nh)]

    with tile.TileContext(nc) as tc:
        with (
            tc.tile_pool(name="const", bufs=1) as constp,
            tc.tile_pool(name="x", bufs=1) as xp,
            tc.tile_pool(name="e", bufs=1) as ep,
            tc.tile_pool(name="y", bufs=1) as yp,
            tc.tile_pool(name="s", bufs=1) as sp_,
            tc.tile_pool(name="psum", bufs=1, space="PSUM") as psump,
        ):
            const_t = constp.tile([NP_, NP_ + 1], BF16, tag="constb")
            ident_t = const_t[:, 0:NP_]
            ones_t = const_t[:, NP_:NP_ + 1]

            xall = xp.tile([NP_, BPP * NF // 2], BF16, tag="xall")
            e = ep.tile([NP_, NBLK * NF], BF16, tag="e")
            y = yp.tile([NP_, 11 * NF], BF16, tag="y")   # P0..P10
            r = sp_.tile([NP_, NF], BF16, tag="r")
            sm_sb = constp.tile([NP_, n_spcols + len(TRACE_P) * NP_],
                                BF16, tag="smsb")

            # PSUM: 2 S banks, 2 SP-col banks, 4 trace banks = 8
            # (warmup shares S bank 1: its writes precede S(1)'s start)
            s_ps = [psump.tile([NP_, 512], FP32, tag=f"s{k}", name=f"s{k}")
                    for k in range(2)]
            spc_ps = psump.tile([NP_, 512], FP32, tag="spc", name="spc")
            spcl_ps = psump.tile([NP_, 512], FP32, tag="spcl", name="spcl")
            m_ps = [psump.tile([NP_, 512], FP32, tag=f"m{k}", name=f"m{k}")
                    for k in range(len(TRACE_P))]
            wu_ps = s_ps[1]

            def ev_(t, nb, h):
                w = ws[h]
                return t[:].rearrange("p (b n) -> p b n", b=nb)[
                    :, :, offs[h]:offs[h] + w]

            def x8v(h):
                w = ws[h]
                c0 = N_X8 * offs[h] // 2
                return xall[:, c0:c0 + N_X8 * w // 2].bitcast(FP8).rearrange(
                    "p (b n) -> p b n", b=N_X8)

            def x16v(h):
                w = ws[h]
                c0 = N_X8 * NF // 2 + N_X16 * offs[h]
                return xall[:, c0:c0 + N_X16 * w].rearrange(
                    "p (b n) -> p b n", b=N_X16)

            def dma_in_span(h0, h1):
                # one x8 DMA + one x16 DMA covering chunks h0..h1-1
                w2 = sum(ws[h0:h1])
                spans = [(N_X8 * offs[h0] // 2, N_X8 * w2 // 2),
                         (N_X8 * NF // 2 + N_X16 * offs[h0], N_X16 * w2)]
                for a, n in spans:
                    src = xall_d[NP_ * a:NP_ * (a + n)].rearrange(
                        "(p n) -> p n", p=NP_, n=n)
                    nc.sync.dma_start(xall[:, a:a + n], src)

            # PE warmup (pstate ramp) runs while input DMAs stream
            wu = cfg["warmup"]
            for i in range(wu):
                nc.tensor.matmul(wu_ps[:, 0:NP_], ident_t, ident_t,
                                 start=(i == 0), stop=(i == wu - 1))

            def front(h):
                w = ws[h]
                ev = ev_(e, NBLK, h)
                nc.scalar.activation(ev[:, ACT_SPAN[0]:ACT_SPAN[1], :],
                                     x8v(h), AF.Exp)
                ei = e[:].bitcast(I16).rearrange("p (b n) -> p b n", b=NBLK)[
                    :, :, offs[h]:offs[h] + w]
                nc.vector.tensor_scalar(
                    out=ei[:, H16_SPAN[0]:H16_SPAN[1], :],
                    in0=x16v(h),
                    scalar1=A1, scalar2=B1, op0=ALU.mult, op1=ALU.add)
                u = unit_of[h]
                sb = s_ps[u % 2]
                uoff = offs[h] - offs[ub[u]]       # col offset within bank
                first = h == ub[u]
                last = h == ub[u + 1] - 1
                # hack-e positions first: they land before ACT's exp
                poss = ([p for p in range(H16_SPAN[0], NBLK) if p != S_SKIP]
                        + list(range(ACT_SPAN[0], ACT_SPAN[1])))
                for i, p in enumerate(poss):
                    nc.tensor.matmul(sb[:, uoff:uoff + w], ident_t,
                                     ev[:, p, :],
                                     start=(first and i == 0),
                                     stop=(last and i == len(poss) - 1))

            def back_dve(u):
                # recip + y + u-out for unit u (DVE/Pool/SP queues)
                off = offs[ub[u]]
                w = sum(ws[ub[u]:ub[u + 1]])
                def bv(t, nb):
                    return t[:].rearrange("p (b n) -> p b n", b=nb)[
                        :, :, off:off + w]
                ev = bv(e, NBLK)
                yv = bv(y, 11)
                rsl = r[:, off:off + w]
                with nc.allow_low_precision("dice: 2e-2 tolerance"):
                    nc.vector.reciprocal(out=rsl, in_=s_ps[u % 2][:, 0:w])
                rb = rsl.unsqueeze(1)
                if u >= nu - ntail:
                    # tail unit: DVE takes P(tp)..P10, Pool P0..P(tp) in
                    # parallel - Pool is idle by now and finishes first
                    tp = (cfg.get("tail_pool", 2) if u == nu - 1
                          else cfg.get("tail_pool0", 2))
                    nc.vector.tensor_tensor(
                        yv[:, tp:11, :], ev[:, tp:11, :],
                        rb.broadcast_to((NP_, 11 - tp, w)), op=ALU.mult)
                    if tp:
                        nc.gpsimd.tensor_tensor(
                            yv[:, 0:tp, :], ev[:, 0:tp, :],
                            rb.broadcast_to((NP_, tp, w)), op=ALU.mult)
                else:
                    nc.vector.tensor_tensor(
                        yv[:, DVE_Y[0]:DVE_Y[1], :],
                        ev[:, DVE_Y[0]:DVE_Y[1], :],
                        rb.broadcast_to((NP_, DVE_Y[1] - DVE_Y[0], w)),
                        op=ALU.mult)
                    nc.gpsimd.tensor_tensor(
                        yv[:, POOL_Y[0]:POOL_Y[1], :],
                        ev[:, POOL_Y[0]:POOL_Y[1], :],
                        rb.broadcast_to((NP_, POOL_Y[1] - POOL_Y[0], w)),
                        op=ALU.mult)
                ug = cfg["u_group"]
                if (u + 1) % ug == 0 or u == nu - 1:
                    u0 = (u // ug) * ug
                    a, bnd = offs[ub[u0]], off + w
                    nc.sync.dma_start(u_d[:, a:bnd],
                                      y[:, 7 * NF + a:7 * NF + bnd])

            def back_pe(u):
                # SP colsums + trace matmuls for unit u (PE queue only)
                off = offs[ub[u]]
                w = sum(ws[ub[u]:ub[u + 1]])
                def bv(t, nb):
                    return t[:].rearrange("p (b n) -> p b n", b=nb)[
                        :, :, off:off + w]
                ev = bv(e, NBLK)
                yv = bv(y, 11)
                tail_u = u >= nu - ntail
                lastu = u == nu - 1
                cps = spcl_ps if lastu else spc_ps
                nj = w // NP_
                for ci in range(N_SPC):
                    col = cps[:, (0 if lastu else u) * N_SPC + ci:
                              (0 if lastu else u) * N_SPC + ci + 1]
                    for j in range(nj):
                        nc.tensor.matmul(
                            col, yv[:, ci, j * NP_:(j + 1) * NP_], ones_t,
                            start=(j == 0), stop=(j == nj - 1))
                if tail_u:
                    # colsum the trace classes' y of this unit instead of
                    # extending the trace groups (closed at nu - ntail - 1).
                    # All tail trace cols live in the spcl bank right after
                    # the last unit's N_SPC cols, matching the tail copy.
                    tb = N_SPC + (u - (nu - ntail)) * len(TRACE_P)
                    for k, tp in enumerate(TRACE_P):
                        col = spcl_ps[:, tb + k:tb + k + 1]
                        for j in range(nj):
                            nc.tensor.matmul(
                                col, yv[:, tp, j * NP_:(j + 1) * NP_],
                                ones_t, start=(j == 0), stop=(j == nj - 1))
                else:
                    for k, tp in enumerate(TRACE_P):
                        mb = m_ps[k]
                        for j in range(nj):
                            sl = slice(j * NP_, (j + 1) * NP_)
                            nc.tensor.matmul(
                                mb[:, 0:NP_],
                                ev[:, tp, sl],
                                r[:, off + j * NP_:off + (j + 1) * NP_],
                                start=(u == 0 and j == 0),
                                stop=(u == nu - ntail - 1 and j == nj - 1))

            dg = cfg.get("dma_group", 1)
            dma_in_span(0, min(dg, nh))
            nc.sync.dma_start(const_t[:], const_dram[:])
            for h in range(dg, nh, dg):
                dma_in_span(h, min(h + dg, nh))
            def m_copies():
                # trace groups closed: copy + ship M early so only the
                # tiny last spc copy sits in the drain path
                for k in range(len(TRACE_P)):
                    dst = sm_sb[:, n_spcols + k * NP_:
                                n_spcols + (k + 1) * NP_]
                    if k % 2 == 0:
                        nc.scalar.activation(dst, m_ps[k][:, 0:NP_],
                                             AF.Copy)
                    else:
                        nc.vector.tensor_copy(dst, m_ps[k][:, 0:NP_])
                nc.sync.dma_start(sm_d[:, n_spcols:], sm_sb[:, n_spcols:])
                if not cfg.get("early_spc", False):
                    return
                # spc cols of units 0..nu-2 are closed too: ship early
                c = N_SPC * (nu - 1)
                nc.scalar.activation(sm_sb[:, 0:c], spc_ps[:, 0:c], AF.Copy)
                nc.sync.dma_start(sm_d[:, 0:c], sm_sb[:, 0:c])

            for h in range(nh):
                front(h)
                # DVE back runs one unit behind the front, the PE back two
                # units behind, so neither in-order queue stalls the next
                # front's work behind a dependency on r/y
                u = unit_of[h]
                if h == ub[u + 1] - 1:
                    if u >= 1:
                        back_dve(u - 1)
                    if u >= 2:
                        back_pe(u - 2)
                        if u - 2 == nu - ntail - 1:
                            m_copies()
            back_dve(nu - 1)
            back_pe(nu - 2)
            if nu - 2 == nu - ntail - 1:
                m_copies()
            back_pe(nu - 1)

            if cfg.get("early_spc", False):
                # only the last unit's spc cols remain (own psum bank)
                c = N_SPC * (nu - 1)
                nlast = n_spcols - c
                nc.vector.tensor_copy(sm_sb[:, c:n_spcols],
                                      spcl_ps[:, 0:nlast])
                nc.sync.dma_start(sm_d[:, c:n_spcols], sm_sb[:, c:n_spcols])
            else:
                c = N_SPC * (nu - 1)
                nlast = n_spcols - c
                nc.vector.tensor_copy(sm_sb[:, 0:c], spc_ps[:, 0:c])
                nc.vector.tensor_copy(sm_sb[:, c:n_spcols],
                                      spcl_ps[:, 0:nlast])
                nc.sync.dma_start(sm_d[:, 0:n_spcols], sm_sb[:, 0:n_spcols])

    _split_dma_waits(nc)
    return nc


def _split_dma_waits(nc):
    """Walrus allows only one sync-wait command per instruction in some
    lowerings. Move all but the last wait onto same-engine no-ops
    inserted right before the instruction."""
    import bass_rust

    builders = {
        mybir.EngineType.Pool: nc.gpsimd,
        mybir.EngineType.SP: nc.sync,
        mybir.EngineType.Activation: nc.scalar,
        mybir.EngineType.DVE: nc.vector,
        mybir.EngineType.PE: nc.tensor,
    }
    f = nc.m.functions[0]
    targets = []
    for b in f.blocks:
        for ins in b.instructions:
            if type(ins).__name__ == "InstNoOp":
                continue
            si = getattr(ins, "sync_info", None)
            if si is not None and len(si.on_wait) > 1 and ins.engine in builders:
                targets.append((b, ins))
    for b, ins in targets:
        si = ins.sync_info
        keep = list(si.on_wait[-1:])
        move = list(si.on_wait[:-1])
        nops = []
        for wv in move:
            nop = builders[ins.engine].nop(nofuse=True).ins
            for b2 in f.blocks:
                lst = b2.instructions
                for j, xx in enumerate(lst):
                    if xx.name == nop.name:
                        del lst[j]
                        break
            nop.sync_info = bass_rust.SyncInfo(on_wait=[wv], on_update=[])
            nops.append(nop)
        ins.sync_info = bass_rust.SyncInfo(on_wait=keep, on_update=si.on_update)
        lst = b.instructions
        idx = next(j for j, xx in enumerate(lst) if xx.name == ins.name)
        for kk, nop in enumerate(nops):
            lst.insert(idx + kk, nop)


_NC_CACHE = None


def _get_nc():
    global _NC_CACHE
    if _NC_CACHE is None:
        _NC_CACHE = build_nc()
    return _NC_CACHE


def _host_prep(logits, targets):
    """Per-core input blob: per chunk h (width w), per partition row:
    [7w bytes fp8 of (c0..c5,c10) | 10w bytes bf16 of (xt,c7,c8,c9,c6)],
    viewed as bf16 and raveled."""
    ws = CFG["ws"]
    lg = logits.reshape(B, C, NP_, NF)            # f32
    tg = targets.reshape(B, NP_, NF)
    xt = np.take_along_axis(lg, tg[:, None, :, :], axis=1)[:, 0]  # [B,128,2048]
    x8_full = np.concatenate(
        [lg[:, 0:6], lg[:, 10:11]], axis=1)       # [B,7,128,2048]
    x16_full = np.stack(
        [xt, lg[:, 7], lg[:, 8], lg[:, 9], lg[:, 6]], axis=1)  # [B,5,...]
    outs = []
    for b in range(B):
        p8s, p16s = [], []
        off = 0
        for w in ws:
            p8s.append(np.ascontiguousarray(
                x8_full[b, :, :, off:off + w].transpose(1, 0, 2)
            ).astype(ml_dtypes.float8_e4m3).view(np.uint8).ravel())
            p16s.append(np.ascontiguousarray(
                x16_full[b, :, :, off:off + w].transpose(1, 0, 2)
            ).astype(ml_dtypes.bfloat16).view(np.uint8).ravel())
            off += w
        blob = np.ascontiguousarray(np.concatenate(p8s + p16s))
        outs.append(blob.view(ml_dtypes.bfloat16))
    return outs, tg


def kernel(logits, targets):
    logits = np.asarray(logits, dtype=np.float32)
    targets = np.asarray(targets).astype(np.int64)

    nc = _get_nc()
    blobs, tg = _host_prep(logits, targets)
    in_maps = [{"xall": blobs[b]} for b in range(B)]

    res = run_bass_kernel_spmd(nc, in_maps, list(range(B)))

    nu = len(CFG["units"])
    n_spcols = N_SPC * nu + len(TRACE_P) * CFG["ntail"]
    npix_core = NP_ * NF
    I = np.zeros(C, np.float64)
    SP = np.zeros(C, np.float64)
    CNT = np.zeros(C, np.float64)
    for b, rr in enumerate(res.results):
        sm = rr["sm_out"].astype(np.float64)       # [128, n_spcols + 384]
        spc = sm[:, 0:N_SPC * nu]
        for ci in range(N_SPC):
            SP[P_CLASS[ci]] += spc[:, ci::N_SPC].sum()
        # tail units' colsums of the trace classes
        ntail = CFG["ntail"]
        for t in range(ntail):
            for k, tp in enumerate(TRACE_P):
                SP[P_CLASS[tp]] += sm[
                    :, N_SPC * nu + t * len(TRACE_P) + k].sum()
        m = sm[:, n_spcols:]                       # [128, 4*128]
        for k, tp in enumerate(TRACE_P):
            SP[P_CLASS[tp]] += np.trace(m[:, k * NP_:(k + 1) * NP_])
        u = rr["u_out"].astype(np.float64).ravel()
        t = tg[b].ravel()
        I += np.bincount(t, weights=u, minlength=C)
        CNT += np.bincount(t, minlength=C)
        SP[6] += npix_core
    SP[6] -= SP[[0, 1, 2, 3, 4, 5, 7, 8, 9, 10]].sum()

    card = SP + CNT
    dice = (2.0 * I + SMOOTH) / (card + SMOOTH)
    return np.float32(1.0 - dice.mean())


# revision 55
# speedup vs baseline: 1.0033x; 1.0033x over previous
"""DiceLoss Trainium2 Bass kernel, v6 (multi-engine softmax split).

Problem: logits [8, 11, 512, 512] f32, targets [8, 512, 512] int.
  probs = softmax(logits, axis=1); I[c] = sum probs[c]*(t==c);
  Card[c] = sum probs[c] + count(t==c); loss = 1 - mean((2I+1)/(Card+1)).

Sharding: data-parallel over batch; core b handles batch element b
(262144 pixels laid out [128 partitions, 2048 free], processed in 8
chunks of 256 pixel-columns).

Per-pixel work is split across all four compute engines:
  e_c = exp(x_c):
    - 7 blocks (c0..c5, c10): exact exp on ACT (fp8 input)
    - 5 blocks (xt, c7..c9, c6): DVE "bit-hack" exp at 4x DVE speed:
      i16 = rne(x*128*log2e + B1) from bf16 input; bitcast as bf16 is
      (1+f)*2^k ~= 2^z (piecewise-linear 2^frac, mean-centered by B1;
      end-to-end error validated ~1e-4 in numsim.py)
  S = sum_c e_c: PE identity-matmul accumulation into PSUM (11 matmuls
    per chunk, hack blocks first since they land earlier; PE is warmed
    up with dummy matmuls so its pstate ramp completes before S(0))
  r = 1/S: DVE reciprocal
  y_c = e_c * r:
    - c0,c1,c2 on Pool (tensor_tensor, stride-0 broadcast r)
    - c3,c4,c5,c10,xt on DVE (one fused tensor_tensor)
    - c7,c8,c9: y never materialized in mid chunks; SP[c] comes from
      the PE "trace trick": M_c += E_slice^T R_slice accumulated over
      128-col slices; diag(M_c) holds per-column sums of e_c*r, host
      sums the diagonal. The last `ntail` chunks materialize y on DVE
      and use column-sum matmuls instead, so the M banks close early
      and their PSUM copies/DMA overlap the drain; tail-unit y splits
      DVE (9 blocks) / Pool (2 blocks) so the drain runs in parallel.
  SP[c] (c0..c5, c10): PE column-sum matmuls (y-slice stationary, ones
    moving, [128,1] psum out: cost ~ 1 moving row, nearly free)
  SP[c6]: residual (sum_c y_c = 1 per pixel => Npix - sum others)
  u = y[xt] (target-class prob of the host-gathered x_t block): DMA'd
    out in chunk pairs; host computes I[c] = bincount(t, u) and
    CNT[c] = bincount(t) (indexing only) plus the final dice ratio.

Pipelining: per chunk, front = DMA/exp/hack/S, back_dve = recip/y/u-out
(one chunk behind), back_pe = colsum/trace matmuls (two behind) - the
split keeps each engine's in-order queue from stalling the next front.
Input rides one bf16-typed blob ([x8 section | x16 section], fp8 part
DMA'd first per chunk); the M matrices ship early, SP columns at the
end. NOTE: reading a PSUM bank shortly after its accumulation group
stops is racy on this stack (the early-spc variant flaked) - keep
PSUM reads well behind their producing matmuls.
TimelineSim: 26320 ns (baseline v5: 35013 ns); HW rel err 4.3e-5,
bit-stable across processes.
"""

import numpy as np
import ml_dtypes

import concourse.bass as bass
import concourse.tile as tile
from concourse import mybir
from concourse.bass_utils import run_bass_kernel_spmd

B, C, H, W = 8, 11, 512, 512
NP_, NF = 128, 2048             # partition x free pixels per core
SMOOTH = 1.0

FP32 = mybir.dt.float32
BF16 = mybir.dt.bfloat16
FP8 = mybir.dt.float8e4
I16 = mybir.dt.int16
AF = mybir.ActivationFunctionType
ALU = mybir.AluOpType

LOG2E = float(np.log2(np.e))
A1 = 128.0 * LOG2E              # z-scale: i16 units per unit x
MEAN_EXCESS = 0.057304959111036226  # E[log2(1+f)-f], f~U[0,1)
B1 = 16256.0 - 128.0 * MEAN_EXCESS  # centers the 2^frac PWL error

# --- e-tile position map --------------------------------------------
# P0..P2:  c0,c1,c2  e=ACT   y=Pool
# P3..P5:  c3,c4,c5  e=ACT   y=DVE
# P6:      c10       e=ACT   y=DVE
# P7:      xt        e=hack  y=DVE -> u out
# P8..P10: c7,c8,c9  e=hack  SP=PE trace (y on DVE for last chunk)
# P11:     c6        e=hack  residual (no y)
NBLK = 12
N_X8 = 7                        # fp8 blocks: [c0..c5, c10] -> P0..P6
N_X16 = 5                       # bf16 blocks: [xt,c7,c8,c9,c6] -> P7..P11
ACT_SPAN = (0, 7)
H16_SPAN = (7, 12)
POOL_Y = (0, 3)
DVE_Y = (3, 8)                  # includes P6 (c10): its y is computed
                                # but unused in mid units (SP via trace)
N_SPC = 7                       # colsum classes P0..P6 (c10's y exists
                                # anyway inside the fused DVE span)
TRACE_P = [8, 9, 10]            # traced positions (c7, c8, c9)
S_SKIP = 7                      # xt not part of S
P_CLASS = [0, 1, 2, 3, 4, 5, 10, -1, 7, 8, 9, 6]  # class of each position
BPP = N_X8 + 2 * N_X16          # input bytes per pixel-column (17)

CFG = dict(
    ws=[256] * 8,               # front (DMA/exp/hack/S) chunk widths
    units=[1] * 8,              # back-stage units, in chunks
    warmup=46,                  # PE warmup matmuls (128 rows each)
    ntail=2,                    # trailing units using colsums (no trace)
    u_group=2,                  # chunks per u-out DMA
)


def build_nc(**over):
    cfg = dict(CFG)
    cfg.update(over)
    ws = cfg["ws"]
    nh = len(ws)
    assert sum(ws) == NF and all(w % 128 == 0 and w <= 512 for w in ws)
    units = cfg["units"]        # chunks per back-unit
    nu = len(units)
    assert sum(units) == nh
    ub = [sum(units[:u]) for u in range(nu + 1)]   # unit chunk bounds
    unit_of = [u for u in range(nu) for _ in range(units[u])]
    ntail = cfg["ntail"]        # units whose trace classes use colsums
    n_spcols = N_SPC * nu + len(TRACE_P) * ntail

    nc = bass.Bass(trn_type="TRN2")

    # input blob (bf16-typed bytes): [whole x8 section | x16 section],
    # each chunk-major, so adjacent chunks merge into one DMA per pair
    xall_d = nc.declare_dram_parameter("xall", [NP_ * BPP * NF // 2], BF16,
                                       isOutput=False)
    u_d = nc.declare_dram_parameter("u_out", [NP_, NF], BF16, isOutput=True)
    sm_d = nc.declare_dram_parameter(
        "sm_out", [NP_, n_spcols + len(TRACE_P) * NP_], BF16, isOutput=True)

    # single const blob: [ident | ones] -> one DMA
    const_np = np.concatenate(
        [np.eye(NP_, dtype=np.float32), np.ones((NP_, 1), np.float32)],
        axis=1).astype(ml_dtypes.bfloat16)
    const_dram = nc.inline_tensor(const_np, name="constb")

    offs = [sum(ws[:h]) for h in range(nh)]

    with tile.TileContext(nc) as tc:
        with (
            tc.tile_pool(name="const", bufs=1) as constp,
            tc.tile_pool(name="x", bufs=1) as xp,
            tc.tile_pool(name="e", bufs=1) as ep,
            tc.tile_pool(name="y", bufs=1) as yp,
            tc.tile_pool(name="s", bufs=1) as sp_,
            tc.tile_pool(name="psum", bufs=1, space="PSUM") as psump,
        ):
            const_t = constp.tile([NP_, NP_ + 1], BF16, tag="constb")
            ident_t = const_t[:, 0:NP_]
            ones_t = const_t[:, NP_:NP_ + 1]

            xall = xp.tile([NP_, BPP * NF // 2], BF16, tag="xall")
            e = ep.tile([NP_, NBLK * NF], BF16, tag="e")
            y = yp.tile([NP_, 11 * NF], BF16, tag="y")   # P0..P10
            r = sp_.tile([NP_, NF], BF16, tag="r")
            sm_sb = constp.tile([NP_, n_spcols + len(TRACE_P) * NP_],
                                BF16, tag="smsb")

            # PSUM: 2 S banks, 2 SP-col banks, 4 trace banks = 8
            # (warmup shares S bank 1: its writes precede S(1)'s start)
            s_ps = [psump.tile([NP_, 512], FP32, tag=f"s{k}", name=f"s{k}")
                    for k in range(2)]
            spc_ps = psump.tile([NP_, 512], FP32, tag="spc", name="spc")
            spcl_ps = psump.tile([NP_, 512], FP32, tag="spcl", name="spcl")
            m_ps = [psump.tile([NP_, 512], FP32, tag=f"m{k}", name=f"m{k}")
                    for k in range(len(TRACE_P))]
            wu_ps = s_ps[1]

            def ev_(t, nb, h):
                w = ws[h]
                return t[:].rearrange("p (b n) -> p b n", b=nb)[
                    :, :, offs[h]:offs[h] + w]

            def x8v(h):
                w = ws[h]
                c0 = N_X8 * offs[h] // 2
                return xall[:, c0:c0 + N_X8 * w // 2].bitcast(FP8).rearrange(
                    "p (b n) -> p b n", b=N_X8)

            def x16v(h):
                w = ws[h]
                c0 = N_X8 * NF // 2 + N_X16 * offs[h]
                return xall[:, c0:c0 + N_X16 * w].rearrange(
                    "p (b n) -> p b n", b=N_X16)

            def dma_in_span(h0, h1):
                # one x8 DMA + one x16 DMA covering chunks h0..h1-1
                w2 = sum(ws[h0:h1])
                spans = [(N_X8 * offs[h0] // 2, N_X8 * w2 // 2),
                         (N_X8 * NF // 2 + N_X16 * offs[h0], N_X16 * w2)]
                for a, n in spans:
                    src = xall_d[NP_ * a:NP_ * (a + n)].rearrange(
                        "(p n) -> p n", p=NP_, n=n)
                    nc.sync.dma_start(xall[:, a:a + n], src)

            # PE warmup (pstate ramp) runs while input DMAs stream
            wu = cfg["warmup"]
            for i in range(wu):
                nc.tensor.matmul(wu_ps[:, 0:NP_], ident_t, ident_t,
                                 start=(i == 0), stop=(i == wu - 1))

            def front(h):
                w = ws[h]
                ev = ev_(e, NBLK, h)
                nc.scalar.activation(ev[:, ACT_SPAN[0]:ACT_SPAN[1], :],
                                     x8v(h), AF.Exp)
                ei = e[:].bitcast(I16).rearrange("p (b n) -> p b n", b=NBLK)[
                    :, :, offs[h]:offs[h] + w]
                nc.vector.tensor_scalar(
                    out=ei[:, H16_SPAN[0]:H16_SPAN[1], :],
                    in0=x16v(h),
                    scalar1=A1, scalar2=B1, op0=ALU.mult, op1=ALU.add)
                u = unit_of[h]
                sb = s_ps[u % 2]
                uoff = offs[h] - offs[ub[u]]       # col offset within bank
                first = h == ub[u]
                last = h == ub[u + 1] - 1
                # hack-e positions first: they land before ACT's exp
                poss = ([p for p in range(H16_SPAN[0], NBLK) if p != S_SKIP]
                        + list(range(ACT_SPAN[0], ACT_SPAN[1])))
                for i, p in enumerate(poss):
                    nc.tensor.matmul(sb[:, uoff:uoff + w], ident_t,
                                     ev[:, p, :],
                                     start=(first and i == 0),
                                     stop=(last and i == len(poss) - 1))

            def back_dve(u):
                # recip + y + u-out for unit u (DVE/Pool/SP queues)
                off = offs[ub[u]]
                w = sum(ws[ub[u]:ub[u + 1]])
                def bv(t, nb):
                    return t[:].rearrange("p (b n) -> p b n", b=nb)[
                        :, :, off:off + w]
                ev = bv(e, NBLK)
                yv = bv(y, 11)
                rsl = r[:, off:off + w]
                with nc.allow_low_precision("dice: 2e-2 tolerance"):
                    nc.vector.reciprocal(out=rsl, in_=s_ps[u % 2][:, 0:w])
                rb = rsl.unsqueeze(1)
                if u >= nu - ntail:
                    # tail unit: DVE takes P(tp)..P10, Pool P0..P(tp) in
                    # parallel - Pool is idle by now and finishes first
                    tp = (cfg.get("tail_pool", 3) if u == nu - 1
                          else cfg.get("tail_pool0", 2))
                    nc.vector.tensor_tensor(
                        yv[:, tp:11, :], ev[:, tp:11, :],
                        rb.broadcast_to((NP_, 11 - tp, w)), op=ALU.mult)
                    if tp:
                        nc.gpsimd.tensor_tensor(
                            yv[:, 0:tp, :], ev[:, 0:tp, :],
                            rb.broadcast_to((NP_, tp, w)), op=ALU.mult)
                else:
                    nc.vector.tensor_tensor(
                        yv[:, DVE_Y[0]:DVE_Y[1], :],
                        ev[:, DVE_Y[0]:DVE_Y[1], :],
                        rb.broadcast_to((NP_, DVE_Y[1] - DVE_Y[0], w)),
                        op=ALU.mult)
                    nc.gpsimd.tensor_tensor(
                        yv[:, POOL_Y[0]:POOL_Y[1], :],
                        ev[:, POOL_Y[0]:POOL_Y[1], :],
                        rb.broadcast_to((NP_, POOL_Y[1] - POOL_Y[0], w)),
                        op=ALU.mult)
                ug = cfg["u_group"]
                if (u + 1) % ug == 0 or u == nu - 1:
                    u0 = (u // ug) * ug
                    a, bnd = offs[ub[u0]], off + w
                    nc.sync.dma_start(u_d[:, a:bnd],
                                      y[:, 7 * NF + a:7 * NF + bnd])

            def back_pe(u):
                # SP colsums + trace matmuls for unit u (PE queue only)
                off = offs[ub[u]]
                w = sum(ws[ub[u]:ub[u + 1]])
                def bv(t, nb):
                    return t[:].rearrange("p (b n) -> p b n", b=nb)[
                        :, :, off:off + w]
                ev = bv(e, NBLK)
                yv = bv(y, 11)
                tail_u = u >= nu - ntail
                lastu = u == nu - 1
                cps = spcl_ps if lastu else spc_ps
                nj = w // NP_
                for ci in range(N_SPC):
                    col = cps[:, (0 if lastu else u) * N_SPC + ci:
                              (0 if lastu else u) * N_SPC + ci + 1]
                    for j in range(nj):
                        nc.tensor.matmul(
                            col, yv[:, ci, j * NP_:(j + 1) * NP_], ones_t,
                            start=(j == 0), stop=(j == nj - 1))
                if tail_u:
                    # colsum the trace classes' y of this unit instead of
                    # extending the trace groups (closed at nu - ntail - 1).
                    # All tail trace cols live in the spcl bank right after
                    # the last unit's N_SPC cols, matching the tail copy.
                    tb = N_SPC + (u - (nu - ntail)) * len(TRACE_P)
                    for k, tp in enumerate(TRACE_P):
                        col = spcl_ps[:, tb + k:tb + k + 1]
                        for j in range(nj):
                            nc.tensor.matmul(
                                col, yv[:, tp, j * NP_:(j + 1) * NP_],
                                ones_t, start=(j == 0), stop=(j == nj - 1))
                else:
                    for k, tp in enumerate(TRACE_P):
                        mb = m_ps[k]
                        for j in range(nj):
                            sl = slice(j * NP_, (j + 1) * NP_)
                            nc.tensor.matmul(
                                mb[:, 0:NP_],
                                ev[:, tp, sl],
                                r[:, off + j * NP_:off + (j + 1) * NP_],
                                start=(u == 0 and j == 0),
                                stop=(u == nu - ntail - 1 and j == nj - 1))

            dg = cfg.get("dma_group", 1)
            dma_in_span(0, min(dg, nh))
            nc.sync.dma_start(const_t[:], const_dram[:])
            for h in range(dg, nh, dg):
                dma_in_span(h, min(h + dg, nh))
            def m_copies():
                # trace groups closed: copy + ship M early so only the
                # tiny last spc copy sits in the drain path
                for k in range(len(TRACE_P)):
                    dst = sm_sb[:, n_spcols + k * NP_:
                                n_spcols + (k + 1) * NP_]
                    if k % 2 == 0:
                        nc.scalar.activation(dst, m_ps[k][:, 0:NP_],
                                             AF.Copy)
                    else:
                        nc.vector.tensor_copy(dst, m_ps[k][:, 0:NP_])
                nc.sync.dma_start(sm_d[:, n_spcols:], sm_sb[:, n_spcols:])
                if not cfg.get("early_spc", False):
                    return
                # spc cols of units 0..nu-2 are closed too: ship early
                c = N_SPC * (nu - 1)
                nc.scalar.activation(sm_sb[:, 0:c], spc_ps[:, 0:c], AF.Copy)
                nc.sync.dma_start(sm_d[:, 0:c], sm_sb[:, 0:c])

            for h in range(nh):
                front(h)
                # DVE back runs one unit behind the front, the PE back two
                # units behind, so neither in-order queue stalls the next
                # front's work behind a dependency on r/y
                u = unit_of[h]
                if h == ub[u + 1] - 1:
                    if u >= 1:
                        back_dve(u - 1)
                    if u >= 2:
                        back_pe(u - 2)
                        if u - 2 == nu - ntail - 1:
                            m_copies()
            back_dve(nu - 1)
            back_pe(nu - 2)
            if nu - 2 == nu - ntail - 1:
                m_copies()
            back_pe(nu - 1)

            if cfg.get("early_spc", False):
                # only the last unit's spc cols remain (own psum bank)
                c = N_SPC * (nu - 1)
                nlast = n_spcols - c
                nc.vector.tensor_copy(sm_sb[:, c:n_spcols],
                                      spcl_ps[:, 0:nlast])
                nc.sync.dma_start(sm_d[:, c:n_spcols], sm_sb[:, c:n_spcols])
            else:
                c = N_SPC * (nu - 1)
                nlast = n_spcols - c
                nc.vector.tensor_copy(sm_sb[:, 0:c], spc_ps[:, 0:c])
                nc.vector.tensor_copy(sm_sb[:, c:n_spcols],
                                      spcl_ps[:, 0:nlast])
                nc.sync.dma_start(sm_d[:, 0:n_spcols], sm_sb[:, 0:n_spcols])

    _split_dma_waits(nc)
    return nc


def _split_dma_waits(nc):
    """Walrus allows only one sync-wait command per instruction in some
    lowerings. Move all but the last wait onto same-engine no-ops
    inserted right before the instruction."""
    import bass_rust

    builders = {
        mybir.EngineType.Pool: nc.gpsimd,
        mybir.EngineType.SP: nc.sync,
        mybir.EngineType.Activation: nc.scalar,
        mybir.EngineType.DVE: nc.vector,
        mybir.EngineType.PE: nc.tensor,
    }
    f = nc.m.functions[0]
    targets = []
    for b in f.blocks:
        for ins in b.instructions:
            if type(ins).__name__ == "InstNoOp":
                continue
            si = getattr(ins, "sync_info", None)
            if si is not None and len(si.on_wait) > 1 and ins.engine in builders:
                targets.append((b, ins))
    for b, ins in targets:
        si = ins.sync_info
        keep = list(si.on_wait[-1:])
        move = list(si.on_wait[:-1])
        nops = []
        for wv in move:
            nop = builders[ins.engine].nop(nofuse=True).ins
            for b2 in f.blocks:
                lst = b2.instructions
                for j, xx in enumerate(lst):
                    if xx.name == nop.name:
                        del lst[j]
                        break
            nop.sync_info = bass_rust.SyncInfo(on_wait=[wv], on_update=[])
            nops.append(nop)
        ins.sync_info = bass_rust.SyncInfo(on_wait=keep, on_update=si.on_update)
        lst = b.instructions
        idx = next(j for j, xx in enumerate(lst) if xx.name == ins.name)
        for kk, nop in enumerate(nops):
            lst.insert(idx + kk, nop)


_NC_CACHE = None


def _get_nc():
    global _NC_CACHE
    if _NC_CACHE is None:
        _NC_CACHE = build_nc()
    return _NC_CACHE


def _host_prep(logits, targets):
    """Per-core input blob: per chunk h (width w), per partition row:
    [7w bytes fp8 of (c0..c5,c10) | 10w bytes bf16 of (xt,c7,c8,c9,c6)],
    viewed as bf16 and raveled."""
    ws = CFG["ws"]
    lg = logits.reshape(B, C, NP_, NF)            # f32
    tg = targets.reshape(B, NP_, NF)
    xt = np.take_along_axis(lg, tg[:, None, :, :], axis=1)[:, 0]  # [B,128,2048]
    x8_full = np.concatenate(
        [lg[:, 0:6], lg[:, 10:11]], axis=1)       # [B,7,128,2048]
    x16_full = np.stack(
        [xt, lg[:, 7], lg[:, 8], lg[:, 9], lg[:, 6]], axis=1)  # [B,5,...]
    outs = []
    for b in range(B):
        p8s, p16s = [], []
        off = 0
        for w in ws:
            p8s.append(np.ascontiguousarray(
                x8_full[b, :, :, off:off + w].transpose(1, 0, 2)
            ).astype(ml_dtypes.float8_e4m3).view(np.uint8).ravel())
            p16s.append(np.ascontiguousarray(
                x16_full[b, :, :, off:off + w].transpose(1, 0, 2)
            ).astype(ml_dtypes.bfloat16).view(np.uint8).ravel())
            off += w
        blob = np.ascontiguousarray(np.concatenate(p8s + p16s))
        outs.append(blob.view(ml_dtypes.bfloat16))
    return outs, tg


def kernel(logits, targets):
    logits = np.asarray(logits, dtype=np.float32)
    targets = np.asarray(targets).astype(np.int64)

    nc = _get_nc()
    blobs, tg = _host_prep(logits, targets)
    in_maps = [{"xall": blobs[b]} for b in range(B)]

    res = run_bass_kernel_spmd(nc, in_maps, list(range(B)))

    nu = len(CFG["units"])
    n_spcols = N_SPC * nu + len(TRACE_P) * CFG["ntail"]
    npix_core = NP_ * NF
    I = np.zeros(C, np.float64)
    SP = np.zeros(C, np.float64)
    CNT = np.zeros(C, np.float64)
    for b, rr in enumerate(res.results):
        sm = rr["sm_out"].astype(np.float64)       # [128, n_spcols + 384]
        spc = sm[:, 0:N_SPC * nu]
        for ci in range(N_SPC):
            SP[P_CLASS[ci]] += spc[:, ci::N_SPC].sum()
        # tail units' colsums of the trace classes
        ntail = CFG["ntail"]
        for t in range(ntail):
            for k, tp in enumerate(TRACE_P):
                SP[P_CLASS[tp]] += sm[
                    :, N_SPC * nu + t * len(TRACE_P) + k].sum()
        m = sm[:, n_spcols:]                       # [128, 4*128]
        for k, tp in enumerate(TRACE_P):
            SP[P_CLASS[tp]] += np.trace(m[:, k * NP_:(k + 1) * NP_])
        u = rr["u_out"].astype(np.float64).ravel()
        t = tg[b].ravel()
        I += np.bincount(t, weights=u, minlength=C)
        CNT += np.bincount(t, minlength=C)
        SP[6] += npix_core
    SP[6] -= SP[[0, 1, 2, 3, 4, 5, 7, 8, 9, 10]].sum()

    card = SP + CNT
    dice = (2.0 * I + SMOOTH) / (card + SMOOTH)
    return np.float32(1.0 - dice.mean())
